# revision 28
# baseline (speedup 1.0000x reference)
"""Trainium2 Bass kernel for nn_BidirectionalBoxPool.

Contract: kernel(x, boxes) takes FULL inputs (x: (8,128,128,128) f32,
boxes: (8,64,4) f32) and returns (feats, widths) matching the reference:
feats (8, 64, 2, 128, 8, MW) f32, widths (8, 64, 2) f32, with MW the
data-dependent max pooled width.

Strategy: data-parallel over the batch axis N — core n handles image n.

Math per image: grid_sample with a per-box separable bilinear grid, so
  feats[k,d,c,i,j] = sum_h sum_w img[c,h,w] * wy_k[h,i] * wx_k[w,j']
with the dir-1 grid an exact (i,j)-flip of dir-0 within each box's valid
width. Host numpy replicates the reference's fp32 grid math exactly and
bakes it into per-image weight tensors (fp16):
  WY  [h=128, NV*8]    y-interp weights, valid wide boxes compacted to
                       the front NV "slots"
  WXF [w=128, NV*128]  x-interp weights: dir0 cols + flipped dir1 cols,
                       zero-padded to 128 cols/slot
Device program (SPMD identical across cores; all box data flows through
the weight tensors, so one compile serves any input with the same
(C, K, MW, NV)):
  stage 1 (y-interp): per channel c: PSUM[w, (slot,i)] = img_c^T @ WY
           -> contiguous cast to SBUF S[w, c*ncols + ki] (fp16, c-major)
  stage 2 (x-interp, swapped operands): per slot: stationary WXF_k
           [w,128] (one LDWEIGHTS), two matmuls with the moving operand
           an S view [[ncols, C/2], [1, 8]] (runs-of-8 strided, full
           rate) -> PSUM [jd=(d,j), (h, c', i)]
  out: per slot, one fp32->fp16 cast to SBUF and a 2KB/partition DMA to
       a DRAM scratch [slot, jd, (h, c', i)]; the host does the final
       (k, d, c, i, j) permutation + fp32 upcast.
PSUM evacuation (casts) is split between the Vector and Scalar engines.
Tall boxes (bh >= bw, ~7%, width<=16) have their grid transposed
relative to the wide layout; they are computed exactly on host, as are
invalid boxes (zeros) and the `widths` output.
"""

from contextlib import ExitStack

import numpy as np

import concourse.bass as bass
import concourse.tile as tile
from concourse import bacc, bass_utils, mybir

F32 = mybir.dt.float32
F32R = mybir.dt.float32r
F16 = mybir.dt.float16

PH = 8
N_CORES = 8
NPF32 = np.float32


# ----------------------------------------------------------------------------
# host-side weight construction (replicates reference fp32 grid math)
# ----------------------------------------------------------------------------

def _box_meta(boxes):
    b = boxes.astype(NPF32)
    xmin, ymin, xmax, ymax = b[:, 0], b[:, 1], b[:, 2], b[:, 3]
    valid = ~((xmin == 0) & (ymin == 0) & (xmax == 0) & (ymax == 0))
    one = NPF32(1.0)
    bw = np.where(valid, (xmax - xmin).astype(NPF32), one).astype(NPF32)
    bh = np.where(valid, (ymax - ymin).astype(NPF32), one).astype(NPF32)
    wide = bw > bh
    ratio = np.where(wide, (bw / bh).astype(NPF32),
                     (bh / bw).astype(NPF32)).astype(NPF32)
    width = np.ceil((ratio * NPF32(PH)).astype(NPF32)).astype(np.int32)
    width = np.where(valid, width, 0)
    wf = np.maximum(width, 2).astype(NPF32)
    return valid, wide, width, wf, bw, bh, xmin.astype(NPF32), ymin.astype(NPF32)


def _max_width(boxes_all):
    b = np.asarray(boxes_all, dtype=np.float64)
    valid = ~np.all(b == 0, axis=-1)
    bw = np.where(valid, b[..., 2] - b[..., 0], 1.0)
    bh = np.where(valid, b[..., 3] - b[..., 1], 1.0)
    ratio = np.where(bw > bh, bw / bh, bh / bw)
    ratio = np.where(valid, ratio, 0.0)
    return int(np.ceil(ratio.max() * PH))


def _grid_wide(xmin, ymin, bw, bh, wf, W, H, ii, jj):
    gx = ((xmin + (jj * bw / (wf - NPF32(1.0))).astype(NPF32)).astype(NPF32)
          - NPF32(W / 2)) / NPF32(W / 2)
    gy = ((ymin + (ii * bh / NPF32(PH - 1.0)).astype(NPF32)).astype(NPF32)
          - NPF32(H / 2)) / NPF32(H / 2)
    return gx.astype(NPF32), gy.astype(NPF32)


def _grid_tall(xmin, ymin, bw, bh, wf, W, H, ii, jj):
    gx = ((xmin + (ii * bw / NPF32(PH - 1.0)).astype(NPF32)).astype(NPF32)
          - NPF32(W / 2)) / NPF32(W / 2)
    gy = ((ymin + ((wf - jj) * bh / (wf - NPF32(1.0))).astype(NPF32)).astype(NPF32)
          - NPF32(H / 2)) / NPF32(H / 2)
    return gx.astype(NPF32), gy.astype(NPF32)


def _taps(g, n):
    g = g.astype(NPF32)
    pos = ((g + NPF32(1.0)) * NPF32(n) - NPF32(1.0)) * NPF32(0.5)
    pos64 = pos.astype(np.float64)
    i0 = np.floor(pos64).astype(np.int64)
    f = pos64 - i0
    w0 = np.where((i0 >= 0) & (i0 <= n - 1), 1.0 - f, 0.0)
    w1 = np.where((i0 + 1 >= 0) & (i0 + 1 <= n - 1), f, 0.0)
    return i0, w0, w1


def _build_image_weights(boxes, H, W, MW):
    K = boxes.shape[0]
    valid, wide, width, wf, bw, bh, xmin, ymin = _box_meta(boxes)
    WY = np.zeros((H, K * PH), np.float64)
    WXF = np.zeros((W, K * 2 * MW), np.float64)
    tall_idx = []
    ii = np.arange(PH, dtype=NPF32)
    for k in range(K):
        if not valid[k]:
            continue
        if not wide[k]:
            tall_idx.append(k)
            continue
        wk = int(width[k])
        jj = np.arange(wk, dtype=NPF32)
        gx, gy = _grid_wide(xmin[k], ymin[k], bw[k], bh[k], wf[k], W, H, ii, jj)
        y0, wy0, wy1 = _taps(gy, H)
        for i in range(PH):
            col = k * PH + i
            if wy0[i] != 0.0:
                WY[y0[i], col] += wy0[i]
            if wy1[i] != 0.0:
                WY[y0[i] + 1, col] += wy1[i]
        x0, wx0, wx1 = _taps(gx, W)
        base = k * 2 * MW
        for j in range(min(wk, MW)):
            if wx0[j] != 0.0:
                WXF[x0[j], base + j] += wx0[j]
            if wx1[j] != 0.0:
                WXF[x0[j] + 1, base + j] += wx1[j]
            jr = wk - 1 - j
            if wx0[jr] != 0.0:
                WXF[x0[jr], base + MW + j] += wx0[jr]
            if wx1[jr] != 0.0:
                WXF[x0[jr] + 1, base + MW + j] += wx1[jr]
    return WY.astype(NPF32), WXF.astype(NPF32), width, tall_idx


def _tall_feats(img, boxes, k, H, W, MW):
    valid, wide, width, wf, bw, bh, xmin, ymin = _box_meta(boxes)
    C = img.shape[0]
    wk = int(width[k])
    out = np.zeros((2, C, PH, MW), NPF32)
    ii = np.arange(PH, dtype=NPF32)[:, None]
    jj = np.arange(wk, dtype=NPF32)[None, :]
    gx, gy = _grid_tall(xmin[k], ymin[k], bw[k], bh[k], wf[k], W, H, ii, jj)
    gx = np.broadcast_to(gx, (PH, wk))
    gy = np.broadcast_to(gy, (PH, wk))
    x0, wx0, wx1 = _taps(gx, W)
    y0, wy0, wy1 = _taps(gy, H)
    imgf = img.astype(np.float64)

    def gat(yc, xc, m):
        yi = np.clip(yc, 0, H - 1)
        xi = np.clip(xc, 0, W - 1)
        return imgf[:, yi, xi] * m

    s = (gat(y0, x0, wy0 * wx0) + gat(y0, x0 + 1, wy0 * wx1)
         + gat(y0 + 1, x0, wy1 * wx0) + gat(y0 + 1, x0 + 1, wy1 * wx1))
    wcl = min(wk, MW)
    out[0, :, :, :wcl] = s[:, :, :wcl].astype(NPF32)
    out[1, :, :, :wcl] = s[:, ::-1, ::-1][:, :, :wcl].astype(NPF32)
    return out


# ----------------------------------------------------------------------------
# device program
# ----------------------------------------------------------------------------

S1_DTYPE = F16  # stage-1 matmul dtype: F16 (fast) or F32R (higher precision)
OUT_F16 = True  # device writes fp16 feats; host upcasts


def _build_program_v4(C, K, MW, NV=None, P=128):
    """v4+: stage-2 swapped — stationary = WXF_k (contiguous, 1 LDW/box),
    moving = S block (c-major sg, runs-of-8 strided AP, full rate).
    Only NV "slots" (valid wide boxes, host-permuted to the front) are
    processed, in two ragged groups; output goes to a DRAM scratch
    [slot, jd, (h, c', i)] (fp16, 2KB/partition DMA chunks) and the final
    (k,d,c,i,j) permutation happens on host."""
    NV = K if NV is None else NV
    NW = 2 * MW
    NWP = 128  # wxf padded to 128 cols/box so DMA tiles have 128 partitions
    assert NW <= NWP
    assert C % 2 == 0
    CH = C // 2  # c-half per N=CH*PH matmul
    kgs = [kg for kg in (min(32, NV), NV - 32) if kg > 0]

    nc = bacc.Bacc("TRN2", target_bir_lowering=False, debug=False,
                   enable_asserts=True, num_devices=1)

    img = nc.dram_tensor("img", [P, C * P], F16, kind="ExternalInput").ap()
    wy = nc.dram_tensor("wy", [P, NV * PH], F16, kind="ExternalInput").ap()
    wxf = nc.dram_tensor("wxf", [P, NV * NWP], F16, kind="ExternalInput").ap()
    feats = nc.dram_tensor("feats", [NV, NWP * C * PH], F16,
                           kind="ExternalOutput").ap()

    with tile.TileContext(nc) as tc, ExitStack() as ctx:
        const_pool = ctx.enter_context(tc.tile_pool(name="const", bufs=1))
        s_pool = ctx.enter_context(tc.tile_pool(name="sg", bufs=2))
        st_pool = ctx.enter_context(tc.tile_pool(name="st", bufs=6))
        ps1_pool = ctx.enter_context(tc.tile_pool(name="ps1", bufs=2, space="PSUM"))
        ps2_pool = ctx.enter_context(tc.tile_pool(name="ps2", bufs=4, space="PSUM"))

        img_t = const_pool.tile([P, C * P], F16)
        wy_t = const_pool.tile([P, NV * PH], F16)
        wxf_t = const_pool.tile([P, NV * NWP], F16)
        nc.sync.dma_start(wy_t[:], wy)
        # img DMA in chunks, smallest first, so stage 1 starts early
        cc = 0
        first_sizes = [4, 4, 8]
        while cc < C:
            CCH = min(first_sizes.pop(0) if first_sizes else 16, C - cc)
            nc.sync.dma_start(img_t[:, cc * P:(cc + CCH) * P],
                              img[:, cc * P:(cc + CCH) * P])
            cc += CCH
        nc.sync.dma_start(wxf_t[:], wxf)  # needed only in stage 2

        slot0 = 0
        for g, KG in enumerate(kgs):
            ncols = KG * PH
            coff = slot0 * PH
            # stage 1: c-major S (free index = c*ncols + ki); two channels
            # share one ps1 tile so casts move 2*ncols columns at a time
            sg = s_pool.tile([P, C * ncols], F16, tag="sg")
            for c4 in range(C // 4):
                ps1 = ps1_pool.tile([P, 4 * ncols], F32, tag="ps1")
                for h in range(4):
                    c = 4 * c4 + h
                    nc.tensor.matmul(
                        ps1[:, h * ncols:(h + 1) * ncols],
                        img_t[:, c * P:(c + 1) * P],
                        wy_t[:, coff:coff + ncols],
                    )
                if c4 % 2 == 0:
                    nc.vector.tensor_copy(
                        sg[:, 4 * c4 * ncols:(4 * c4 + 4) * ncols], ps1[:])
                else:
                    nc.scalar.copy(
                        sg[:, 4 * c4 * ncols:(4 * c4 + 4) * ncols], ps1[:])

            # stage 2 (swapped): per slot, one LDW + 2 matmuls N=CH*PH
            for kl in range(KG):
                k = slot0 + kl
                st = st_pool.tile([NWP, C * PH], F16, tag="st")
                for h in range(2):
                    ps2 = ps2_pool.tile([NWP, CH * PH], F32, tag="ps2")
                    mov = bass.AP(sg.tensor,
                                  sg[:].offset + h * CH * ncols + kl * PH,
                                  [[sg[:].ap[0][0], P], [ncols, CH], [1, PH]])
                    nc.tensor.matmul(
                        ps2[:], wxf_t[:, k * NWP:k * NWP + NWP], mov)
                    dst = st[:, h * CH * PH:(h + 1) * CH * PH]
                    # both copy engines work the same box in parallel
                    if (kl + h) % 2 == 0:
                        nc.vector.tensor_copy(dst, ps2[:])
                    else:
                        nc.scalar.copy(dst, ps2[:])
                nc.sync.dma_start(feats[k], st[:])
            slot0 += KG

    nc.compile()
    return nc


_PROGRAM_CACHE = {}


def _get_program(C, K, MW, NV):
    key = (C, K, MW, NV)
    if key not in _PROGRAM_CACHE:
        _PROGRAM_CACHE[key] = _build_program_v4(C, K, MW, NV=NV)
    return _PROGRAM_CACHE[key]


# ----------------------------------------------------------------------------
# entry point
# ----------------------------------------------------------------------------

def kernel(x, boxes, _run_kwargs=None):
    x = np.asarray(x, dtype=np.float32)
    boxes = np.asarray(boxes, dtype=np.float32)
    N, C, H, W = x.shape
    K = boxes.shape[1]
    assert N == N_CORES and H == 128 and W == 128 and C == 128

    MW = _max_width(boxes)
    CH = C // 2

    # per-image weights, with valid wide boxes compacted to the front slots
    per_image = []
    for n in range(N):
        WY, WXF, width, tall_idx = _build_image_weights(boxes[n], H, W, MW)
        valid, wide, _, _, _, _, _, _ = _box_meta(boxes[n])
        perm = [k for k in range(K) if valid[k] and wide[k]]
        per_image.append((WY, WXF, width, tall_idx, perm))
    NV = max(len(p[4]) for p in per_image)

    nc = _get_program(C, K, MW, NV)

    in_maps = []
    for n in range(N):
        WY, WXF, width, tall_idx, perm = per_image[n]
        img = np.ascontiguousarray(
            x[n].transpose(1, 0, 2).reshape(H, C * W))  # [h, (c, w)]
        WYs = np.zeros((H, NV * PH), np.float16)
        WXFP = np.zeros((W, NV * 128), np.float16)
        for s, k in enumerate(perm):
            WYs[:, s * PH:(s + 1) * PH] = WY[:, k * PH:(k + 1) * PH]
            WXFP[:, s * 128:s * 128 + 2 * MW] = WXF[:, k * 2 * MW:(k + 1) * 2 * MW]
        in_maps.append({
            "img": img.astype(np.float16),
            "wy": WYs,
            "wxf": WXFP,
        })

    res = bass_utils.run_bass_kernel_spmd(
        nc, in_maps, core_ids=list(range(N_CORES)), **(_run_kwargs or {}))

    feats = np.zeros((N, K, 2, C, PH, MW), np.float32)
    widths = np.empty((N, K, 2), np.float32)
    for n in range(N):
        WY, WXF, width, tall_idx, perm = per_image[n]
        nv = len(perm)
        # device scratch layout: [slot, jd, (h, c', i)] with c = h*CH + c';
        # jd = d*MW + j (rows 2*MW..127 are padding); the i axis is the
        # stationary row, so the dir1 half is i-flipped
        s = res.results[n]["feats"].reshape(NV, 128, 2, CH, PH)
        s = s[:nv, :2 * MW].reshape(nv, 2, MW, 2, CH, PH)
        t = s.transpose(0, 1, 3, 4, 5, 2)  # (slot, d, h, c', i, j)
        t = np.concatenate([t[:, :1], t[:, 1:, :, :, ::-1, :]], axis=1)
        feats[n, perm] = t.reshape(nv, 2, C, PH, MW).astype(np.float32)
        for k in tall_idx:
            feats[n, k] = _tall_feats(x[n], boxes[n], k, H, W, MW)
        widths[n] = width.astype(np.float32)[:, None]
    kernel.last_result = res
    return feats, widths


# revision 30
# speedup vs baseline: 1.1769x; 1.1769x over previous
"""Trainium2 Bass kernel for nn_BidirectionalBoxPool.

Contract: kernel(x, boxes) takes FULL inputs (x: (8,128,128,128) f32,
boxes: (8,64,4) f32) and returns (feats, widths) matching the reference:
feats (8, 64, 2, 128, 8, MW) f32, widths (8, 64, 2) f32, with MW the
data-dependent max pooled width.

Strategy: data-parallel over the batch axis N — core n handles image n.

Math per image: grid_sample with a per-box separable bilinear grid, so
  feats[k,d,c,i,j] = sum_h sum_w img[c,h,w] * wy_k[h,i] * wx_k[w,j']
with the dir-1 grid an exact (i,j)-flip of dir-0 within each box's valid
width. Host numpy replicates the reference's fp32 grid math exactly and
bakes it into per-image weight tensors (fp16):
  WY  [h=128, NV*8]    y-interp weights, valid wide boxes compacted to
                       the front NV "slots"
  WXF [w=128, NV*128]  x-interp weights: dir0 cols + flipped dir1 cols,
                       zero-padded to 128 cols/slot
Device program (SPMD identical across cores; all box data flows through
the weight tensors, so one compile serves any input with the same
(C, K, MW, NV)):
  stage 1 (y-interp): per channel c: PSUM[w, (slot,i)] = img_c^T @ WY
           -> contiguous cast to SBUF S[w, c*ncols + ki] (fp16, c-major)
  stage 2 (x-interp, swapped operands): per slot: stationary WXF_k
           [w,128] (one LDWEIGHTS), two matmuls with the moving operand
           an S view [[ncols, C/2], [1, 8]] (runs-of-8 strided, full
           rate) -> PSUM [jd=(d,j), (h, c', i)]
  out: per slot, one fp32->fp16 cast to SBUF and a 2KB/partition DMA to
       a DRAM scratch [slot, jd, (h, c', i)]; the host does the final
       (k, d, c, i, j) permutation + fp32 upcast.
PSUM evacuation (casts) is split between the Vector and Scalar engines.
Tall boxes (bh >= bw, ~7%, width<=16) have their grid transposed
relative to the wide layout; they are computed exactly on host, as are
invalid boxes (zeros) and the `widths` output.
"""

from contextlib import ExitStack

import numpy as np

import concourse.bass as bass
import concourse.tile as tile
from concourse import bacc, bass_utils, mybir

F32 = mybir.dt.float32
F32R = mybir.dt.float32r
F16 = mybir.dt.float16

PH = 8
N_CORES = 8
NPF32 = np.float32


# ----------------------------------------------------------------------------
# host-side weight construction (replicates reference fp32 grid math)
# ----------------------------------------------------------------------------

def _box_meta(boxes):
    b = boxes.astype(NPF32)
    xmin, ymin, xmax, ymax = b[:, 0], b[:, 1], b[:, 2], b[:, 3]
    valid = ~((xmin == 0) & (ymin == 0) & (xmax == 0) & (ymax == 0))
    one = NPF32(1.0)
    bw = np.where(valid, (xmax - xmin).astype(NPF32), one).astype(NPF32)
    bh = np.where(valid, (ymax - ymin).astype(NPF32), one).astype(NPF32)
    wide = bw > bh
    ratio = np.where(wide, (bw / bh).astype(NPF32),
                     (bh / bw).astype(NPF32)).astype(NPF32)
    width = np.ceil((ratio * NPF32(PH)).astype(NPF32)).astype(np.int32)
    width = np.where(valid, width, 0)
    wf = np.maximum(width, 2).astype(NPF32)
    return valid, wide, width, wf, bw, bh, xmin.astype(NPF32), ymin.astype(NPF32)


def _max_width(boxes_all):
    b = np.asarray(boxes_all, dtype=np.float64)
    valid = ~np.all(b == 0, axis=-1)
    bw = np.where(valid, b[..., 2] - b[..., 0], 1.0)
    bh = np.where(valid, b[..., 3] - b[..., 1], 1.0)
    ratio = np.where(bw > bh, bw / bh, bh / bw)
    ratio = np.where(valid, ratio, 0.0)
    return int(np.ceil(ratio.max() * PH))


def _grid_wide(xmin, ymin, bw, bh, wf, W, H, ii, jj):
    gx = ((xmin + (jj * bw / (wf - NPF32(1.0))).astype(NPF32)).astype(NPF32)
          - NPF32(W / 2)) / NPF32(W / 2)
    gy = ((ymin + (ii * bh / NPF32(PH - 1.0)).astype(NPF32)).astype(NPF32)
          - NPF32(H / 2)) / NPF32(H / 2)
    return gx.astype(NPF32), gy.astype(NPF32)


def _grid_tall(xmin, ymin, bw, bh, wf, W, H, ii, jj):
    gx = ((xmin + (ii * bw / NPF32(PH - 1.0)).astype(NPF32)).astype(NPF32)
          - NPF32(W / 2)) / NPF32(W / 2)
    gy = ((ymin + ((wf - jj) * bh / (wf - NPF32(1.0))).astype(NPF32)).astype(NPF32)
          - NPF32(H / 2)) / NPF32(H / 2)
    return gx.astype(NPF32), gy.astype(NPF32)


def _taps(g, n):
    g = g.astype(NPF32)
    pos = ((g + NPF32(1.0)) * NPF32(n) - NPF32(1.0)) * NPF32(0.5)
    pos64 = pos.astype(np.float64)
    i0 = np.floor(pos64).astype(np.int64)
    f = pos64 - i0
    w0 = np.where((i0 >= 0) & (i0 <= n - 1), 1.0 - f, 0.0)
    w1 = np.where((i0 + 1 >= 0) & (i0 + 1 <= n - 1), f, 0.0)
    return i0, w0, w1


def _build_image_weights(boxes, H, W, MW):
    K = boxes.shape[0]
    valid, wide, width, wf, bw, bh, xmin, ymin = _box_meta(boxes)
    WY = np.zeros((H, K * PH), np.float64)
    WXF = np.zeros((W, K * 2 * MW), np.float64)
    tall_idx = []
    ii = np.arange(PH, dtype=NPF32)
    for k in range(K):
        if not valid[k]:
            continue
        if not wide[k]:
            tall_idx.append(k)
            continue
        wk = int(width[k])
        jj = np.arange(wk, dtype=NPF32)
        gx, gy = _grid_wide(xmin[k], ymin[k], bw[k], bh[k], wf[k], W, H, ii, jj)
        y0, wy0, wy1 = _taps(gy, H)
        for i in range(PH):
            col = k * PH + i
            if wy0[i] != 0.0:
                WY[y0[i], col] += wy0[i]
            if wy1[i] != 0.0:
                WY[y0[i] + 1, col] += wy1[i]
        x0, wx0, wx1 = _taps(gx, W)
        base = k * 2 * MW
        for j in range(min(wk, MW)):
            if wx0[j] != 0.0:
                WXF[x0[j], base + j] += wx0[j]
            if wx1[j] != 0.0:
                WXF[x0[j] + 1, base + j] += wx1[j]
            jr = wk - 1 - j
            if wx0[jr] != 0.0:
                WXF[x0[jr], base + MW + j] += wx0[jr]
            if wx1[jr] != 0.0:
                WXF[x0[jr] + 1, base + MW + j] += wx1[jr]
    return WY.astype(NPF32), WXF.astype(NPF32), width, tall_idx


def _tall_feats(img, boxes, k, H, W, MW):
    valid, wide, width, wf, bw, bh, xmin, ymin = _box_meta(boxes)
    C = img.shape[0]
    wk = int(width[k])
    out = np.zeros((2, C, PH, MW), NPF32)
    ii = np.arange(PH, dtype=NPF32)[:, None]
    jj = np.arange(wk, dtype=NPF32)[None, :]
    gx, gy = _grid_tall(xmin[k], ymin[k], bw[k], bh[k], wf[k], W, H, ii, jj)
    gx = np.broadcast_to(gx, (PH, wk))
    gy = np.broadcast_to(gy, (PH, wk))
    x0, wx0, wx1 = _taps(gx, W)
    y0, wy0, wy1 = _taps(gy, H)
    imgf = img.astype(np.float64)

    def gat(yc, xc, m):
        yi = np.clip(yc, 0, H - 1)
        xi = np.clip(xc, 0, W - 1)
        return imgf[:, yi, xi] * m

    s = (gat(y0, x0, wy0 * wx0) + gat(y0, x0 + 1, wy0 * wx1)
         + gat(y0 + 1, x0, wy1 * wx0) + gat(y0 + 1, x0 + 1, wy1 * wx1))
    wcl = min(wk, MW)
    out[0, :, :, :wcl] = s[:, :, :wcl].astype(NPF32)
    out[1, :, :, :wcl] = s[:, ::-1, ::-1][:, :, :wcl].astype(NPF32)
    return out


# ----------------------------------------------------------------------------
# device program
# ----------------------------------------------------------------------------

S1_DTYPE = F16  # stage-1 matmul dtype: F16 (fast) or F32R (higher precision)
OUT_F16 = True  # device writes fp16 feats; host upcasts


def _build_program_v4(C, K, MW, NV=None, P=128):
    """v4+: stage-2 swapped — stationary = WXF_k (contiguous, 1 LDW/box),
    moving = S block (c-major sg, runs-of-8 strided AP, full rate).
    Only NV "slots" (valid wide boxes, host-permuted to the front) are
    processed, in two ragged groups; output goes to a DRAM scratch
    [slot, jd, (h, c', i)] (fp16, 2KB/partition DMA chunks) and the final
    (k,d,c,i,j) permutation happens on host."""
    NV = K if NV is None else NV
    NW = 2 * MW
    NWP = 128  # wxf padded to 128 cols/box so DMA tiles have 128 partitions
    assert NW <= NWP
    assert C % 2 == 0
    CH = C // 2  # c-half per N=CH*PH matmul
    kgs = [kg for kg in (min(32, NV), NV - 32) if kg > 0]

    nc = bacc.Bacc("TRN2", target_bir_lowering=False, debug=False,
                   enable_asserts=True, num_devices=1)

    img = nc.dram_tensor("img", [P, C * P], F16, kind="ExternalInput").ap()
    wy = nc.dram_tensor("wy", [P, NV * PH], F16, kind="ExternalInput").ap()
    wxf = nc.dram_tensor("wxf", [P, NV * NWP], F16, kind="ExternalInput").ap()
    feats = nc.dram_tensor("feats", [NV, NWP * C * PH], F16,
                           kind="ExternalOutput").ap()

    with tile.TileContext(nc) as tc, ExitStack() as ctx:
        const_pool = ctx.enter_context(tc.tile_pool(name="const", bufs=1))
        s_pool = ctx.enter_context(tc.tile_pool(name="sg", bufs=2))
        st_pool = ctx.enter_context(tc.tile_pool(name="st", bufs=8))
        ps1_pool = ctx.enter_context(tc.tile_pool(name="ps1", bufs=4, space="PSUM"))
        ps2_pool = ctx.enter_context(tc.tile_pool(name="ps2", bufs=4, space="PSUM"))

        wy_t = const_pool.tile([P, NV * PH], F16)
        wxf_t = const_pool.tile([P, NV * NWP], F16)
        nc.sync.dma_start(wy_t[:], wy)
        # img lives in separate per-chunk tiles: Tile dependencies are
        # per-tile, so stage 1 can start once the FIRST chunk lands
        CCH = min(8, C)
        img_ts = []
        for j, cc in enumerate(range(0, C, CCH)):
            t = const_pool.tile([P, CCH * P], F16, tag=f"img{j}")
            nc.sync.dma_start(t[:], img[:, cc * P:(cc + CCH) * P])
            img_ts.append(t)
        nc.sync.dma_start(wxf_t[:], wxf)  # needed only in stage 2

        slot0 = 0
        for g, KG in enumerate(kgs):
            ncols = KG * PH
            coff = slot0 * PH
            # stage 1: c-major S (free index = c*ncols + ki); two channels
            # share one ps1 tile so casts move 2*ncols columns at a time
            sg = s_pool.tile([P, C * ncols], F16, tag="sg")
            for c2 in range(C // 2):
                ps1 = ps1_pool.tile([P, 2 * ncols], F32, tag="ps1")
                for h in range(2):
                    c = 2 * c2 + h
                    nc.tensor.matmul(
                        ps1[:, h * ncols:(h + 1) * ncols],
                        img_ts[c // CCH][:, (c % CCH) * P:(c % CCH + 1) * P],
                        wy_t[:, coff:coff + ncols],
                    )
                if c2 % 2 == 0:
                    nc.vector.tensor_copy(
                        sg[:, 2 * c2 * ncols:(2 * c2 + 2) * ncols], ps1[:])
                else:
                    nc.scalar.copy(
                        sg[:, 2 * c2 * ncols:(2 * c2 + 2) * ncols], ps1[:])

            # stage 2 (swapped): per slot, one LDW + 2 matmuls N=CH*PH
            for kl in range(KG):
                k = slot0 + kl
                st = st_pool.tile([NWP, C * PH], F16, tag="st")
                for h in range(2):
                    ps2 = ps2_pool.tile([NWP, CH * PH], F32, tag="ps2")
                    mov = bass.AP(sg.tensor,
                                  sg[:].offset + h * CH * ncols + kl * PH,
                                  [[sg[:].ap[0][0], P], [ncols, CH], [1, PH]])
                    nc.tensor.matmul(
                        ps2[:], wxf_t[:, k * NWP:k * NWP + NWP], mov)
                    dst = st[:, h * CH * PH:(h + 1) * CH * PH]
                    # both copy engines work the same box in parallel
                    if (kl + h) % 2 == 0:
                        nc.vector.tensor_copy(dst, ps2[:])
                    else:
                        nc.scalar.copy(dst, ps2[:])
                nc.sync.dma_start(feats[k], st[:])
            slot0 += KG

    nc.compile()
    return nc


_PROGRAM_CACHE = {}


def _get_program(C, K, MW, NV):
    key = (C, K, MW, NV)
    if key not in _PROGRAM_CACHE:
        _PROGRAM_CACHE[key] = _build_program_v4(C, K, MW, NV=NV)
    return _PROGRAM_CACHE[key]


# ----------------------------------------------------------------------------
# entry point
# ----------------------------------------------------------------------------

def kernel(x, boxes, _run_kwargs=None):
    x = np.asarray(x, dtype=np.float32)
    boxes = np.asarray(boxes, dtype=np.float32)
    N, C, H, W = x.shape
    K = boxes.shape[1]
    assert N == N_CORES and H == 128 and W == 128 and C == 128

    MW = _max_width(boxes)
    CH = C // 2

    # per-image weights, with valid wide boxes compacted to the front slots
    per_image = []
    for n in range(N):
        WY, WXF, width, tall_idx = _build_image_weights(boxes[n], H, W, MW)
        valid, wide, _, _, _, _, _, _ = _box_meta(boxes[n])
        perm = [k for k in range(K) if valid[k] and wide[k]]
        per_image.append((WY, WXF, width, tall_idx, perm))
    NV = max(len(p[4]) for p in per_image)

    nc = _get_program(C, K, MW, NV)

    in_maps = []
    for n in range(N):
        WY, WXF, width, tall_idx, perm = per_image[n]
        img = np.ascontiguousarray(
            x[n].transpose(1, 0, 2).reshape(H, C * W))  # [h, (c, w)]
        WYs = np.zeros((H, NV * PH), np.float16)
        WXFP = np.zeros((W, NV * 128), np.float16)
        for s, k in enumerate(perm):
            WYs[:, s * PH:(s + 1) * PH] = WY[:, k * PH:(k + 1) * PH]
            WXFP[:, s * 128:s * 128 + 2 * MW] = WXF[:, k * 2 * MW:(k + 1) * 2 * MW]
        in_maps.append({
            "img": img.astype(np.float16),
            "wy": WYs,
            "wxf": WXFP,
        })

    res = bass_utils.run_bass_kernel_spmd(
        nc, in_maps, core_ids=list(range(N_CORES)), **(_run_kwargs or {}))

    feats = np.zeros((N, K, 2, C, PH, MW), np.float32)
    widths = np.empty((N, K, 2), np.float32)
    for n in range(N):
        WY, WXF, width, tall_idx, perm = per_image[n]
        nv = len(perm)
        # device scratch layout: [slot, jd, (h, c', i)] with c = h*CH + c';
        # jd = d*MW + j (rows 2*MW..127 are padding); the i axis is the
        # stationary row, so the dir1 half is i-flipped
        s = res.results[n]["feats"].reshape(NV, 128, 2, CH, PH)
        s = s[:nv, :2 * MW].reshape(nv, 2, MW, 2, CH, PH)
        t = s.transpose(0, 1, 3, 4, 5, 2)  # (slot, d, h, c', i, j)
        t = np.concatenate([t[:, :1], t[:, 1:, :, :, ::-1, :]], axis=1)
        feats[n, perm] = t.reshape(nv, 2, C, PH, MW).astype(np.float32)
        for k in tall_idx:
            feats[n, k] = _tall_feats(x[n], boxes[n], k, H, W, MW)
        widths[n] = width.astype(np.float32)[:, None]
    kernel.last_result = res
    return feats, widths
